# revision 22
# baseline (speedup 1.0000x reference)
"""2-layer GCN (PyG GCNConv semantics) on 8 Trainium2 NeuronCores.

Strategy (dst-sharded message passing, v2):
  - Nodes are split into 8 contiguous blocks of 6250 rows; core c owns output
    rows [6250c, 6250(c+1)).  Edges (plus one self-loop per node) are
    partitioned by destination core, then by 256-node destination windows,
    then packed into 128-edge tiles with EXACT per-window tile counts.
  - Layer 1: per window, two dma_gather calls (lo: src < 32768 against the
    base x table, hi: the rest against an offset view; int16 index limit),
    512-byte descriptors.  Queue pairs alternate across windows so all four
    SWDGE queues (4 Q7 core pairs) generate descriptors concurrently --
    measured descriptor floor ~3.3 ns/row at 512B on 4 queues vs ~8.2 on one.
  - Scatter: per edge tile one DVE tensor_scalar builds the norm-scaled
    one-hot dst matrix (sel = (iota == dst_local) * norm; padded lanes have
    dst_local=-1 and are killed), and the PE accumulates
    aggT[feat, dst] += msg^T @ sel in float32r.
  - Window flush: h1T = relu(W1^T @ aggT + b1); pT = W2^T @ h1T; transpose to
    row-major and store p rows (f32r) to a per-chunk DRAM buffer.
  - AllGather runs in 2 chunks overlapped with the layer-1 tail.  p_full is
    laid out chunk-major ([chunk][core][local rows]) so both collective
    endpoints are contiguous; the host precomputes the node -> table-row map.
  - Layer 2 gathers PAIRS of 256-byte p rows as single 512-byte descriptors
    (table viewed as [25000, 128] f32): descriptor rate for 256B rows is
    measurably worse (~4.1 ns/row), and pairing also removes the lo/hi
    split.  Edge tiles are segregated by source-row parity; the scatter
    matmul reads the matching 64-column half of the gathered pair.

Host-side work is index preprocessing only (degrees/norms from edge_index,
sorting, packing); all FLOPs on the gathered/aggregated features run on
device.
"""

import os
import sys

import numpy as np

for _p in ("/opt/trn_rl_repo", "/root/.axon_site/_ro/trn_rl_repo"):
    if os.path.isdir(_p) and _p not in sys.path:
        sys.path.insert(0, _p)

import concourse.bacc as bacc
import concourse.tile as tile
from concourse import mybir
from concourse.bass_utils import run_bass_kernel_spmd

P = 128
N_NODES = 50000
C_IN = 128
C_HID = 128
C_OUT = 64
CORES = 8
BLOCK = N_NODES // CORES          # 6250
WIN = 256                         # dst nodes per PSUM window
NW = -(-BLOCK // WIN)             # 25 windows per core (last has 106 rows)
SPLIT = 32768                     # int16-addressable base-table rows
GAT_BUFS = 6                      # msg tile buffering depth
ACT_EVERY = 2                     # every Nth sel tile on Act (0 = off)
ABL_LITE_SCATTER = bool(int(os.environ.get("ABL_LITE_SCATTER", "0")))
ABL_NO_GATHER = bool(int(os.environ.get("ABL_NO_GATHER", "0")))
ABL_NO_COLL = bool(int(os.environ.get("ABL_NO_COLL", "0")))
CHUNK_WINS = (NW,)               # allgather chunks


def _chunk_geometry():
    rows = []
    r0 = 0
    bases = []
    base = 0
    for cw in CHUNK_WINS:
        rk = min(BLOCK - r0, cw * WIN)
        rows.append(rk)
        bases.append(base)
        base += rk * CORES
        r0 += rk
    return rows, bases


CHUNK_ROWS, CHUNK_BASES = _chunk_geometry()


def _tab_map():
    """node id -> chunk-major row in p_full ([chunk][core][local row])."""
    n = np.arange(N_NODES, dtype=np.int64)
    q = n // BLOCK
    r = n % BLOCK
    tab = np.empty(N_NODES, dtype=np.int64)
    r0 = 0
    for rk, base in zip(CHUNK_ROWS, CHUNK_BASES):
        m = (r >= r0) & (r < r0 + rk)
        tab[m] = base + q[m] * rk + (r[m] - r0)
        r0 += rk
    return tab


def _pack_tiles(vals_idx, vals_dloc, vals_nrm, n_tiles):
    """Pad a call's edge list to n_tiles*128 lanes.  Padding gathers row 0
    (idx=0, a real address -- keeps every msg lane finite without memsets)
    and is killed in the selection matrix via dloc=-1 / nrm=0."""
    n = len(vals_idx)
    idx = np.full(n_tiles * P, -1, np.int16)
    dl = np.full(n_tiles * P, -1.0, np.float32)
    nm = np.zeros(n_tiles * P, np.float32)
    idx[:n] = vals_idx
    dl[:n] = vals_dloc
    nm[:n] = vals_nrm
    return idx, dl, nm


def _wrap16(a):
    """[L] int16 idx array -> [128, L/16] wrapped+replicated layout."""
    w16 = a.reshape(-1, 16).T.reshape(16, -1)
    return np.tile(w16, (8, 1))


def _preprocess(edge_index):
    src = np.asarray(edge_index[0], dtype=np.int64)
    dst = np.asarray(edge_index[1], dtype=np.int64)

    deg = np.bincount(dst, minlength=N_NODES).astype(np.float64) + 1.0
    dinv = 1.0 / np.sqrt(deg)

    a_src = np.concatenate([src, np.arange(N_NODES, dtype=np.int64)])
    a_dst = np.concatenate([dst, np.arange(N_NODES, dtype=np.int64)])
    a_nrm = np.concatenate([dinv[src] * dinv[dst], dinv * dinv]).astype(np.float32)

    tab = _tab_map()
    core = a_dst // BLOCK
    win = (a_dst % BLOCK) // WIN
    dloc = ((a_dst % BLOCK) % WIN).astype(np.float32)

    per_core = []
    for c in range(CORES):
        wins = []
        for w in range(NW):
            m = (core == c) & (win == w)
            s = a_src[m]
            d = dloc[m]
            nm = a_nrm[m]

            # layer 1: split by int16 addressability
            lo = s < SPLIT
            n_lo, n_hi = int(lo.sum()), int((~lo).sum())
            t_lo, t_hi = -(-n_lo // P), -(-n_hi // P)
            i_lo, dl_lo, nm_lo = _pack_tiles(s[lo].astype(np.int16), d[lo], nm[lo], t_lo)
            i_hi, dl_hi, nm_hi = _pack_tiles((s[~lo] - SPLIT).astype(np.int16),
                                             d[~lo], nm[~lo], t_hi)

            # layer 2: pair-gather from chunk-major table; split by parity
            t2 = tab[s]
            ev = (t2 & 1) == 0
            n_e, n_o = int(ev.sum()), int((~ev).sum())
            t_e, t_o = -(-n_e // P), -(-n_o // P)
            i_e, dl_e, nm_e = _pack_tiles((t2[ev] >> 1).astype(np.int16), d[ev], nm[ev], t_e)
            i_o, dl_o, nm_o = _pack_tiles((t2[~ev] >> 1).astype(np.int16), d[~ev], nm[~ev], t_o)

            wins.append({
                "cnt1": (n_lo, n_hi), "t1": (t_lo, t_hi),
                "idx1": np.concatenate([i_lo, i_hi]),
                "dl1": np.concatenate([dl_lo, dl_hi]),
                "nm1": np.concatenate([nm_lo, nm_hi]),
                "cnt2": (n_e, n_o), "t2": (t_e, t_o),
                "idx2": np.concatenate([i_e, i_o]),
                "dl2": np.concatenate([dl_e, dl_o]),
                "nm2": np.concatenate([nm_e, nm_o]),
            })
        per_core.append(wins)

    sig = tuple(
        (wins[w]["t1"], wins[w]["t2"]) for wins in per_core for w in range(NW)
    )
    return sig, per_core


_BUILD_CACHE = {}


def _build(sig, per_core0):
    """per_core0: any core's window list -- only tile-count structure is used,
    and it must be identical across cores, so callers pass the max-shaped
    structure via _uniformize."""
    if sig in _BUILD_CACHE:
        return _BUILD_CACHE[sig]

    # per-window tile counts (uniform across cores after _uniformize)
    T1 = [sum(w["t1"]) for w in per_core0]
    T2 = [sum(w["t2"]) for w in per_core0]
    TB1 = np.concatenate([[0], np.cumsum(T1)]).astype(int)   # tile col bases
    TB2 = np.concatenate([[0], np.cumsum(T2)]).astype(int)
    tot1, tot2 = int(TB1[-1]), int(TB2[-1])
    t1max, t2max = max(T1), max(T2)

    # meta f32 columns:
    # [dl1 | nm1 | ndl1 | mnm1 | dl2 | nm2 | ndl2 | mnm2 | iota(WIN) | b1 | b2]
    c_dl1, c_nm1, c_ndl1, c_mnm1 = 0, tot1, 2 * tot1, 3 * tot1
    base2 = 4 * tot1
    c_dl2, c_nm2 = base2, base2 + tot2
    c_ndl2, c_mnm2 = base2 + 2 * tot2, base2 + 3 * tot2
    c_iota = base2 + 4 * tot2
    c_b1, c_b2 = c_iota + WIN, c_iota + WIN + 1
    meta_cols = c_iota + WIN + 2

    f32, f32r, i16 = mybir.dt.float32, mybir.dt.float32r, mybir.dt.int16
    RELU = mybir.ActivationFunctionType.Relu
    COPY = mybir.ActivationFunctionType.Copy
    IDENT = mybir.ActivationFunctionType.Identity

    nc = bacc.Bacc("TRN2", num_devices=CORES, num_swdge_queues=4)
    x_ext = nc.dram_tensor("x", [N_NODES, C_IN], f32r, kind="ExternalInput")
    i1_ext = nc.dram_tensor("idx1", [P, tot1 * 8], i16, kind="ExternalInput")
    i2_ext = nc.dram_tensor("idx2", [P, tot2 * 8], i16, kind="ExternalInput")
    meta_ext = nc.dram_tensor("meta", [P, meta_cols], f32, kind="ExternalInput")
    cnt_ext = nc.dram_tensor("cnt", [1, NW * 4], mybir.dt.int32, kind="ExternalInput")
    w_ext = nc.dram_tensor("wts", [P, 256], f32r, kind="ExternalInput")
    out_ext = nc.dram_tensor("out", [BLOCK, C_OUT], f32, kind="ExternalOutput")

    with tile.TileContext(nc) as tc:
        with tc.tile_pool(name="const", bufs=1) as cpool, \
             tc.tile_pool(name="work", bufs=4) as wpool, \
             tc.tile_pool(name="flush", bufs=2) as fpool, \
             tc.tile_pool(name="dram", bufs=1, space="DRAM") as dpool, \
             tc.tile_pool(name="ps_agg", bufs=2, space="PSUM") as ps_agg, \
             tc.tile_pool(name="ps_z", bufs=2, space="PSUM") as ps_z, \
             tc.tile_pool(name="ps_pt", bufs=2, space="PSUM") as ps_pt, \
             tc.tile_pool(name="ps_rm", bufs=2, space="PSUM") as ps_rm:

            i1_s = cpool.tile([P, tot1 * 8], i16)
            i2_s = cpool.tile([P, tot2 * 8], i16)
            meta_s = cpool.tile([P, meta_cols], f32)
            w_s = cpool.tile([P, 256], f32r)
            cnt_s = cpool.tile([1, NW * 4], mybir.dt.int32)
            nc.sync.dma_start(out=cnt_s[:], in_=cnt_ext[:])
            nc.sync.dma_start(out=i1_s[:], in_=i1_ext[:])
            nc.sync.dma_start(out=i2_s[:], in_=i2_ext[:])
            nc.sync.dma_start(out=meta_s[:], in_=meta_ext[:])
            nc.sync.dma_start(out=w_s[:], in_=w_ext[:])

            p_mine = [dpool.tile([rk, C_OUT], f32r, name=f"p_mine{rk}")
                      for rk in CHUNK_ROWS]
            p_full = dpool.tile([N_NODES // 2, 2 * C_OUT], f32r)

            def build_sel(col, c_dl, c_nm, c_ndl, c_mnm, on_act):
                sel = wpool.tile([P, WIN], f32r, tag="sel")
                if on_act:
                    u = wpool.tile([P, WIN], f32, tag="selu")
                    nc.scalar.activation(
                        out=u[:], in_=meta_s[:, c_iota:c_iota + WIN],
                        func=mybir.ActivationFunctionType.Abs,
                        bias=meta_s[:, c_ndl + col:c_ndl + col + 1])
                    nc.scalar.activation(
                        out=sel[:], in_=u[:], func=RELU,
                        scale=meta_s[:, c_mnm + col:c_mnm + col + 1],
                        bias=meta_s[:, c_nm + col:c_nm + col + 1])
                else:
                    nc.vector.tensor_scalar(
                        out=sel[:],
                        in0=meta_s[:, c_iota:c_iota + WIN],
                        scalar1=meta_s[:, c_dl + col:c_dl + col + 1],
                        scalar2=meta_s[:, c_nm + col:c_nm + col + 1],
                        op0=mybir.AluOpType.is_equal,
                        op1=mybir.AluOpType.mult,
                    )
                return sel

            def store_rowmajor(w, colT_s, dram_dst, row_off, dt_out):
                rows = min(WIN, BLOCK - w * WIN)
                for h in range((rows + P - 1) // P):
                    rh = min(P, rows - h * P)
                    rm = ps_rm.tile([P, C_OUT], f32r, space="PSUM", tag="rm")
                    nc.tensor.transpose(
                        out=rm[:],
                        in_=colT_s[:, h * P:(h + 1) * P],
                        identity=w_s[0:C_OUT, 192:256],
                    )
                    rm_s = fpool.tile([P, C_OUT], dt_out, tag="rm_s")
                    nc.scalar.activation(out=rm_s[:], in_=rm[:].bitcast(f32), func=COPY)
                    r0 = w * WIN + h * P - row_off
                    nc.sync.dma_start(out=dram_dst[r0:r0 + rh, :], in_=rm_s[:rh, :])

            # ---------------- layer 1 ----------------
            import contextlib
            _es = contextlib.ExitStack()
            _rctx = contextlib.ExitStack()
            ra = _rctx.enter_context(nc.gpsimd.register("ra"))
            rb = _rctx.enter_context(nc.gpsimd.register("rb"))

            chunk_of = []
            acc_w = 0
            for k, cw in enumerate(CHUNK_WINS):
                chunk_of += [k] * cw

            gat1 = _es.enter_context(tc.tile_pool(name="gat1", bufs=GAT_BUFS))
            for w in range(NW):
                t_lo, t_hi = per_core0[w]["t1"]
                t_w = t_lo + t_hi
                msg = gat1.tile([P, t1max, C_IN], f32r, tag="msg1")
                if w < GAT_BUFS:
                    nc.vector.memset(msg[:].rearrange("p c e -> p (c e)").bitcast(f32), 0.0)
                ib = int(TB1[w]) * 8
                nc.gpsimd.reg_load(ra, cnt_s[0:1, 4 * w:4 * w + 1])
                nc.gpsimd.reg_load(rb, cnt_s[0:1, 4 * w + 1:4 * w + 2])
                if t_lo and not ABL_NO_GATHER:
                    nc.gpsimd.dma_gather(
                        out_ap=msg[:, 0:t_lo, :], in_ap=x_ext[:],
                        idxs_ap=i1_s[:, ib:ib + t_lo * 8],
                        num_idxs=t_lo * P, num_idxs_reg=ra, elem_size=C_IN,
                        single_packet=False, queue_num=(w % 2) * 2,
                    )
                if t_hi and not ABL_NO_GATHER:
                    nc.gpsimd.dma_gather(
                        out_ap=msg[:, t_lo:t_w, :], in_ap=x_ext[SPLIT:, :],
                        idxs_ap=i1_s[:, ib + t_lo * 8:ib + t_w * 8],
                        num_idxs=t_hi * P, num_idxs_reg=rb, elem_size=C_IN,
                        single_packet=False, queue_num=(w % 2) * 2 + 1,
                    )
                agg = ps_agg.tile([P, WIN], f32, space="PSUM", tag="agg")
                for t in range(1 if ABL_LITE_SCATTER else t_w):
                    col = int(TB1[w]) + t
                    sel = build_sel(col, c_dl1, c_nm1, c_ndl1, c_mnm1,
                                    on_act=(ACT_EVERY > 0 and col % ACT_EVERY == 0))
                    nc.tensor.matmul(
                        out=agg[:, :], lhsT=msg[:, t, :], rhs=sel[:],
                        start=(t == 0),
                        stop=(t == (0 if ABL_LITE_SCATTER else t_w - 1)),
                    )

                agg_s = wpool.tile([P, WIN], f32r, tag="agg_s")
                nc.scalar.activation(out=agg_s[:], in_=agg[:], func=COPY)
                z = ps_z.tile([P, WIN], f32, space="PSUM", tag="z")
                nc.tensor.matmul(out=z[:], lhsT=w_s[:, 0:C_HID], rhs=agg_s[:],
                                 start=True, stop=True)
                h1_s = wpool.tile([P, WIN], f32r, tag="h1")
                nc.scalar.activation(out=h1_s[:], in_=z[:], func=RELU,
                                     bias=meta_s[:, c_b1:c_b1 + 1])
                pt = ps_pt.tile([C_OUT, WIN], f32, space="PSUM", tag="pt")
                nc.tensor.matmul(out=pt[:], lhsT=w_s[:, 128:128 + C_OUT],
                                 rhs=h1_s[:], start=True, stop=True)
                pt_s = fpool.tile([C_OUT, WIN], f32r, tag="pt_s")
                nc.scalar.activation(out=pt_s[:], in_=pt[:], func=COPY)
                k = chunk_of[w]
                row_off = sum(CHUNK_ROWS[:k])
                store_rowmajor(w, pt_s, p_mine[k], row_off, f32r)

                # fire the allgather chunk as soon as its last window stored
                if (w == NW - 1 or chunk_of[w + 1] != k) and not ABL_NO_COLL:
                    pb = CHUNK_BASES[k] // 2
                    pr = CHUNK_ROWS[k] * CORES // 2
                    nc.gpsimd.collective_compute(
                        "AllGather", mybir.AluOpType.bypass,
                        replica_groups=[list(range(CORES))],
                        ins=[p_mine[k][:]], outs=[p_full[pb:pb + pr, :]],
                    )

            # ---------------- layer 2 ----------------
            _es.close()
            _es2 = contextlib.ExitStack()
            gat2 = _es2.enter_context(tc.tile_pool(name="gat2", bufs=GAT_BUFS))
            for w in range(NW):
                t_e, t_o = per_core0[w]["t2"]
                t_w = t_e + t_o
                msg = gat2.tile([P, t2max, 2 * C_OUT], f32r, tag="msg2")
                if w < GAT_BUFS:
                    nc.vector.memset(msg[:].rearrange("p c e -> p (c e)").bitcast(f32), 0.0)
                ib = int(TB2[w]) * 8
                nc.gpsimd.reg_load(ra, cnt_s[0:1, 4 * w + 2:4 * w + 3])
                nc.gpsimd.reg_load(rb, cnt_s[0:1, 4 * w + 3:4 * w + 4])
                if t_e and not ABL_NO_GATHER:
                    nc.gpsimd.dma_gather(
                        out_ap=msg[:, 0:t_e, :], in_ap=p_full[:],
                        idxs_ap=i2_s[:, ib:ib + t_e * 8],
                        num_idxs=t_e * P, num_idxs_reg=ra, elem_size=2 * C_OUT,
                        single_packet=False, queue_num=(w % 2) * 2,
                    )
                if t_o and not ABL_NO_GATHER:
                    nc.gpsimd.dma_gather(
                        out_ap=msg[:, t_e:t_w, :], in_ap=p_full[:],
                        idxs_ap=i2_s[:, ib + t_e * 8:ib + t_w * 8],
                        num_idxs=t_o * P, num_idxs_reg=rb, elem_size=2 * C_OUT,
                        single_packet=False, queue_num=(w % 2) * 2 + 1,
                    )
                agg = ps_agg.tile([P, WIN], f32, space="PSUM", tag="agg")
                for t in range(1 if ABL_LITE_SCATTER else t_w):
                    col = int(TB2[w]) + t
                    half = 0 if t < t_e else C_OUT
                    sel = build_sel(col, c_dl2, c_nm2, c_ndl2, c_mnm2,
                                    on_act=(ACT_EVERY > 0 and col % ACT_EVERY == 0))
                    nc.tensor.matmul(
                        out=agg[:C_OUT, :], lhsT=msg[:, t, half:half + C_OUT],
                        rhs=sel[:],
                        start=(t == 0),
                        stop=(t == (0 if ABL_LITE_SCATTER else t_w - 1)),
                    )

                o_s = fpool.tile([C_OUT, WIN], f32r, tag="o_s")
                nc.scalar.activation(out=o_s[:], in_=agg[:C_OUT, :], func=IDENT,
                                     bias=meta_s[0:C_OUT, c_b2:c_b2 + 1])
                store_rowmajor(w, o_s, out_ext, 0, f32)
            _es2.close()

    _rctx.close()
    nc.compile()
    layout = (c_dl1, c_nm1, c_ndl1, c_mnm1, c_dl2, c_nm2, c_ndl2, c_mnm2,
              c_iota, c_b1, c_b2, meta_cols, TB1, TB2)
    _BUILD_CACHE[sig] = (nc, layout)
    return nc, layout


def _uniformize(per_core):
    """Pad every core/window to the max tile counts so one program serves all
    cores (SPMD).  Returns the padded per-core data plus the shared structure."""
    uni = []
    for w in range(NW):
        t_lo = max(pc[w]["t1"][0] for pc in per_core)
        t_hi = max(pc[w]["t1"][1] for pc in per_core)
        t_e = max(pc[w]["t2"][0] for pc in per_core)
        t_o = max(pc[w]["t2"][1] for pc in per_core)
        uni.append({"t1": (t_lo, t_hi), "t2": (t_e, t_o)})

    padded = []
    for pc in per_core:
        wins = []
        for w in range(NW):
            src_w = pc[w]
            out_w = {"cnt1": src_w["cnt1"], "cnt2": src_w["cnt2"],
                     "t1": uni[w]["t1"], "t2": uni[w]["t2"]}
            for (layer, tkey) in (("1", "t1"), ("2", "t2")):
                ta_s, tb_s = src_w[tkey]
                ta_u, tb_u = uni[w][tkey]
                idx = np.full((ta_u + tb_u) * P, -1, np.int16)
                dl = np.full((ta_u + tb_u) * P, -1.0, np.float32)
                nm = np.zeros((ta_u + tb_u) * P, np.float32)
                s_idx, s_dl, s_nm = src_w["idx" + layer], src_w["dl" + layer], src_w["nm" + layer]
                idx[:ta_s * P] = s_idx[:ta_s * P]
                dl[:ta_s * P] = s_dl[:ta_s * P]
                nm[:ta_s * P] = s_nm[:ta_s * P]
                idx[ta_u * P:ta_u * P + tb_s * P] = s_idx[ta_s * P:]
                dl[ta_u * P:ta_u * P + tb_s * P] = s_dl[ta_s * P:]
                nm[ta_u * P:ta_u * P + tb_s * P] = s_nm[ta_s * P:]
                out_w["idx" + layer] = idx
                out_w["dl" + layer] = dl
                out_w["nm" + layer] = nm
            wins.append(out_w)
        padded.append(wins)
    return uni, padded


def _make_inputs(x, W1, b1, W2, b2, uni, padded, layout):
    (c_dl1, c_nm1, c_ndl1, c_mnm1, c_dl2, c_nm2, c_ndl2, c_mnm2,
     c_iota, c_b1, c_b2, meta_cols, TB1, TB2) = layout
    tot1, tot2 = int(TB1[-1]), int(TB2[-1])

    wts = np.zeros((P, 256), np.float32)
    wts[:, 0:128] = W1
    wts[:128, 128:192] = W2
    wts[0:64, 192:256] = np.eye(64, dtype=np.float32)

    def lanes(flat):  # [T*P] -> [P, T] (tile t, lane p) at column t
        return flat.reshape(-1, P).T

    in_maps = []
    for pc in padded:
        idx1 = np.concatenate([pc[w]["idx1"] for w in range(NW)])
        idx2 = np.concatenate([pc[w]["idx2"] for w in range(NW)])
        dl1 = np.concatenate([pc[w]["dl1"] for w in range(NW)])
        nm1 = np.concatenate([pc[w]["nm1"] for w in range(NW)])
        dl2 = np.concatenate([pc[w]["dl2"] for w in range(NW)])
        nm2 = np.concatenate([pc[w]["nm2"] for w in range(NW)])

        meta = np.zeros((P, meta_cols), np.float32)
        meta[:, c_dl1:c_dl1 + tot1] = lanes(dl1)
        meta[:, c_nm1:c_nm1 + tot1] = lanes(nm1)
        meta[:, c_ndl1:c_ndl1 + tot1] = -lanes(dl1)
        meta[:, c_mnm1:c_mnm1 + tot1] = -lanes(nm1)
        meta[:, c_dl2:c_dl2 + tot2] = lanes(dl2)
        meta[:, c_nm2:c_nm2 + tot2] = lanes(nm2)
        meta[:, c_ndl2:c_ndl2 + tot2] = -lanes(dl2)
        meta[:, c_mnm2:c_mnm2 + tot2] = -lanes(nm2)
        meta[:, c_iota:c_iota + WIN] = np.arange(WIN, dtype=np.float32)[None, :]
        meta[:, c_b1] = b1
        meta[:C_OUT, c_b2] = b2

        cnt = np.zeros((NW, 4), np.int32)
        for w in range(NW):
            cnt[w, 0], cnt[w, 1] = pc[w]["cnt1"]
            cnt[w, 2], cnt[w, 3] = pc[w]["cnt2"]

        in_maps.append({
            "x": np.ascontiguousarray(x, dtype=np.float32),
            "idx1": _wrap16(idx1),
            "idx2": _wrap16(idx2),
            "meta": meta,
            "wts": wts,
            "cnt": cnt.reshape(1, -1),
        })
    return in_maps


def kernel(x, edge_index, W1, b1, W2, b2):
    x = np.asarray(x, dtype=np.float32)
    W1 = np.asarray(W1, dtype=np.float32)
    b1 = np.asarray(b1, dtype=np.float32)
    W2 = np.asarray(W2, dtype=np.float32)
    b2 = np.asarray(b2, dtype=np.float32)

    sig, per_core = _preprocess(np.asarray(edge_index))
    uni, padded = _uniformize(per_core)
    usig = tuple((u["t1"], u["t2"]) for u in uni)
    nc, layout = _build(usig, uni)
    in_maps = _make_inputs(x, W1, b1, W2, b2, uni, padded, layout)
    res = run_bass_kernel_spmd(nc, in_maps, list(range(CORES)))
    out = np.concatenate([res.results[c]["out"] for c in range(CORES)], axis=0)
    return out.astype(np.float32)


# revision 23
# speedup vs baseline: 1.1045x; 1.1045x over previous
"""2-layer GCN (PyG GCNConv semantics) on 8 Trainium2 NeuronCores.

Strategy (dst-sharded message passing, v2):
  - Nodes are split into 8 contiguous blocks of 6250 rows; core c owns output
    rows [6250c, 6250(c+1)).  Edges (plus one self-loop per node) are
    partitioned by destination core, then by 256-node destination windows,
    then packed into 128-edge tiles with EXACT per-window tile counts.
  - Layer 1: per window, two dma_gather calls (lo: src < 32768 against the
    base x table, hi: the rest against an offset view; int16 index limit),
    512-byte descriptors.  Queue pairs alternate across windows so all four
    SWDGE queues (4 Q7 core pairs) generate descriptors concurrently --
    measured descriptor floor ~3.3 ns/row at 512B on 4 queues vs ~8.2 on one.
  - Scatter: per edge tile one DVE tensor_scalar builds the norm-scaled
    one-hot dst matrix (sel = (iota == dst_local) * norm; padded lanes have
    dst_local=-1 and are killed), and the PE accumulates
    aggT[feat, dst] += msg^T @ sel in float32r.
  - Window flush: h1T = relu(W1^T @ aggT + b1); pT = W2^T @ h1T; transpose to
    row-major and store p rows (f32r) to a per-chunk DRAM buffer.
  - AllGather runs in 2 chunks overlapped with the layer-1 tail.  p_full is
    laid out chunk-major ([chunk][core][local rows]) so both collective
    endpoints are contiguous; the host precomputes the node -> table-row map.
  - Layer 2 gathers PAIRS of 256-byte p rows as single 512-byte descriptors
    (table viewed as [25000, 128] f32): descriptor rate for 256B rows is
    measurably worse (~4.1 ns/row), and pairing also removes the lo/hi
    split.  Edge tiles are segregated by source-row parity; the scatter
    matmul reads the matching 64-column half of the gathered pair.

Host-side work is index preprocessing only (degrees/norms from edge_index,
sorting, packing); all FLOPs on the gathered/aggregated features run on
device.
"""

import os
import sys

import numpy as np

for _p in ("/opt/trn_rl_repo", "/root/.axon_site/_ro/trn_rl_repo"):
    if os.path.isdir(_p) and _p not in sys.path:
        sys.path.insert(0, _p)

import concourse.bacc as bacc
import concourse.tile as tile
from concourse import mybir
from concourse.bass_utils import run_bass_kernel_spmd

P = 128
N_NODES = 50000
C_IN = 128
C_HID = 128
C_OUT = 64
CORES = 8
BLOCK = N_NODES // CORES          # 6250
WIN = 256                         # dst nodes per PSUM window
NW = -(-BLOCK // WIN)             # 25 windows per core (last has 106 rows)
SPLIT = 32768                     # int16-addressable base-table rows
GAT_BUFS = 6                      # msg tile buffering depth
ACT_EVERY = 3                     # every Nth sel tile on Act (0 = off)
ABL_LITE_SCATTER = bool(int(os.environ.get("ABL_LITE_SCATTER", "0")))
ABL_NO_GATHER = bool(int(os.environ.get("ABL_NO_GATHER", "0")))
ABL_NO_COLL = bool(int(os.environ.get("ABL_NO_COLL", "0")))
CHUNK_WINS = (NW,)               # allgather chunks


def _chunk_geometry():
    rows = []
    r0 = 0
    bases = []
    base = 0
    for cw in CHUNK_WINS:
        rk = min(BLOCK - r0, cw * WIN)
        rows.append(rk)
        bases.append(base)
        base += rk * CORES
        r0 += rk
    return rows, bases


CHUNK_ROWS, CHUNK_BASES = _chunk_geometry()


def _tab_map():
    """node id -> chunk-major row in p_full ([chunk][core][local row])."""
    n = np.arange(N_NODES, dtype=np.int64)
    q = n // BLOCK
    r = n % BLOCK
    tab = np.empty(N_NODES, dtype=np.int64)
    r0 = 0
    for rk, base in zip(CHUNK_ROWS, CHUNK_BASES):
        m = (r >= r0) & (r < r0 + rk)
        tab[m] = base + q[m] * rk + (r[m] - r0)
        r0 += rk
    return tab


def _pack_tiles(vals_idx, vals_dloc, vals_nrm, n_tiles):
    """Pad a call's edge list to n_tiles*128 lanes.  Padding gathers row 0
    (idx=0, a real address -- keeps every msg lane finite without memsets)
    and is killed in the selection matrix via dloc=-1 / nrm=0."""
    n = len(vals_idx)
    idx = np.full(n_tiles * P, -1, np.int16)
    dl = np.full(n_tiles * P, -1.0, np.float32)
    nm = np.zeros(n_tiles * P, np.float32)
    idx[:n] = vals_idx
    dl[:n] = vals_dloc
    nm[:n] = vals_nrm
    return idx, dl, nm


def _wrap16(a):
    """[L] int16 idx array -> [128, L/16] wrapped+replicated layout."""
    w16 = a.reshape(-1, 16).T.reshape(16, -1)
    return np.tile(w16, (8, 1))


def _preprocess(edge_index):
    src = np.asarray(edge_index[0], dtype=np.int64)
    dst = np.asarray(edge_index[1], dtype=np.int64)

    deg = np.bincount(dst, minlength=N_NODES).astype(np.float64) + 1.0
    dinv = 1.0 / np.sqrt(deg)

    a_src = np.concatenate([src, np.arange(N_NODES, dtype=np.int64)])
    a_dst = np.concatenate([dst, np.arange(N_NODES, dtype=np.int64)])
    a_nrm = np.concatenate([dinv[src] * dinv[dst], dinv * dinv]).astype(np.float32)

    tab = _tab_map()
    core = a_dst // BLOCK
    win = (a_dst % BLOCK) // WIN
    dloc = ((a_dst % BLOCK) % WIN).astype(np.float32)

    per_core = []
    for c in range(CORES):
        wins = []
        for w in range(NW):
            m = (core == c) & (win == w)
            s = a_src[m]
            d = dloc[m]
            nm = a_nrm[m]

            # layer 1: split by int16 addressability
            lo = s < SPLIT
            n_lo, n_hi = int(lo.sum()), int((~lo).sum())
            t_lo, t_hi = -(-n_lo // P), -(-n_hi // P)
            i_lo, dl_lo, nm_lo = _pack_tiles(s[lo].astype(np.int16), d[lo], nm[lo], t_lo)
            i_hi, dl_hi, nm_hi = _pack_tiles((s[~lo] - SPLIT).astype(np.int16),
                                             d[~lo], nm[~lo], t_hi)

            # layer 2: pair-gather from chunk-major table; split by parity
            t2 = tab[s]
            ev = (t2 & 1) == 0
            n_e, n_o = int(ev.sum()), int((~ev).sum())
            t_e, t_o = -(-n_e // P), -(-n_o // P)
            i_e, dl_e, nm_e = _pack_tiles((t2[ev] >> 1).astype(np.int16), d[ev], nm[ev], t_e)
            i_o, dl_o, nm_o = _pack_tiles((t2[~ev] >> 1).astype(np.int16), d[~ev], nm[~ev], t_o)

            wins.append({
                "cnt1": (n_lo, n_hi), "t1": (t_lo, t_hi),
                "idx1": np.concatenate([i_lo, i_hi]),
                "dl1": np.concatenate([dl_lo, dl_hi]),
                "nm1": np.concatenate([nm_lo, nm_hi]),
                "cnt2": (n_e, n_o), "t2": (t_e, t_o),
                "idx2": np.concatenate([i_e, i_o]),
                "dl2": np.concatenate([dl_e, dl_o]),
                "nm2": np.concatenate([nm_e, nm_o]),
            })
        per_core.append(wins)

    sig = tuple(
        (wins[w]["t1"], wins[w]["t2"]) for wins in per_core for w in range(NW)
    )
    return sig, per_core


_BUILD_CACHE = {}


def _build(sig, per_core0):
    """per_core0: any core's window list -- only tile-count structure is used,
    and it must be identical across cores, so callers pass the max-shaped
    structure via _uniformize."""
    if sig in _BUILD_CACHE:
        return _BUILD_CACHE[sig]

    # per-window tile counts (uniform across cores after _uniformize)
    T1 = [sum(w["t1"]) for w in per_core0]
    T2 = [sum(w["t2"]) for w in per_core0]
    TB1 = np.concatenate([[0], np.cumsum(T1)]).astype(int)   # tile col bases
    TB2 = np.concatenate([[0], np.cumsum(T2)]).astype(int)
    tot1, tot2 = int(TB1[-1]), int(TB2[-1])
    t1max, t2max = max(T1), max(T2)

    # meta f32 columns:
    # [dl1 | nm1 | ndl1 | mnm1 | dl2 | nm2 | ndl2 | mnm2 | iota(WIN) | b1 | b2]
    c_dl1, c_nm1, c_ndl1, c_mnm1 = 0, tot1, 2 * tot1, 3 * tot1
    base2 = 4 * tot1
    c_dl2, c_nm2 = base2, base2 + tot2
    c_ndl2, c_mnm2 = base2 + 2 * tot2, base2 + 3 * tot2
    c_iota = base2 + 4 * tot2
    c_b1, c_b2 = c_iota + WIN, c_iota + WIN + 1
    meta_cols = c_iota + WIN + 2

    f32, f32r, i16 = mybir.dt.float32, mybir.dt.float32r, mybir.dt.int16
    RELU = mybir.ActivationFunctionType.Relu
    COPY = mybir.ActivationFunctionType.Copy
    IDENT = mybir.ActivationFunctionType.Identity

    nc = bacc.Bacc("TRN2", num_devices=CORES, num_swdge_queues=4)
    x_ext = nc.dram_tensor("x", [N_NODES, C_IN], f32r, kind="ExternalInput")
    i1_ext = nc.dram_tensor("idx1", [P, tot1 * 8], i16, kind="ExternalInput")
    i2_ext = nc.dram_tensor("idx2", [P, tot2 * 8], i16, kind="ExternalInput")
    meta_ext = nc.dram_tensor("meta", [P, meta_cols], f32, kind="ExternalInput")
    cnt_ext = nc.dram_tensor("cnt", [1, NW * 4], mybir.dt.int32, kind="ExternalInput")
    w_ext = nc.dram_tensor("wts", [P, 256], f32r, kind="ExternalInput")
    out_ext = nc.dram_tensor("out", [BLOCK, C_OUT], f32, kind="ExternalOutput")

    with tile.TileContext(nc) as tc:
        with tc.tile_pool(name="const", bufs=1) as cpool, \
             tc.tile_pool(name="work", bufs=4) as wpool, \
             tc.tile_pool(name="flush", bufs=2) as fpool, \
             tc.tile_pool(name="dram", bufs=1, space="DRAM") as dpool, \
             tc.tile_pool(name="ps_agg", bufs=2, space="PSUM") as ps_agg, \
             tc.tile_pool(name="ps_z", bufs=2, space="PSUM") as ps_z, \
             tc.tile_pool(name="ps_pt", bufs=2, space="PSUM") as ps_pt, \
             tc.tile_pool(name="ps_rm", bufs=2, space="PSUM") as ps_rm:

            i1_s = cpool.tile([P, tot1 * 8], i16)
            i2_s = cpool.tile([P, tot2 * 8], i16)
            meta_s = cpool.tile([P, meta_cols], f32)
            w_s = cpool.tile([P, 256], f32r)
            cnt_s = cpool.tile([1, NW * 4], mybir.dt.int32)
            nc.sync.dma_start(out=cnt_s[:], in_=cnt_ext[:])
            nc.sync.dma_start(out=i1_s[:], in_=i1_ext[:])
            nc.sync.dma_start(out=i2_s[:], in_=i2_ext[:])
            nc.sync.dma_start(out=meta_s[:], in_=meta_ext[:])
            nc.sync.dma_start(out=w_s[:], in_=w_ext[:])

            p_mine = [dpool.tile([rk, C_OUT], f32r, name=f"p_mine{rk}")
                      for rk in CHUNK_ROWS]
            p_full = dpool.tile([N_NODES // 2, 2 * C_OUT], f32r)

            def build_sel(col, c_dl, c_nm, c_ndl, c_mnm, on_act):
                sel = wpool.tile([P, WIN], f32r, tag="sel")
                if on_act:
                    u = wpool.tile([P, WIN], f32, tag="selu")
                    nc.scalar.activation(
                        out=u[:], in_=meta_s[:, c_iota:c_iota + WIN],
                        func=mybir.ActivationFunctionType.Abs,
                        bias=meta_s[:, c_ndl + col:c_ndl + col + 1])
                    nc.scalar.activation(
                        out=sel[:], in_=u[:], func=RELU,
                        scale=meta_s[:, c_mnm + col:c_mnm + col + 1],
                        bias=meta_s[:, c_nm + col:c_nm + col + 1])
                else:
                    nc.vector.tensor_scalar(
                        out=sel[:],
                        in0=meta_s[:, c_iota:c_iota + WIN],
                        scalar1=meta_s[:, c_dl + col:c_dl + col + 1],
                        scalar2=meta_s[:, c_nm + col:c_nm + col + 1],
                        op0=mybir.AluOpType.is_equal,
                        op1=mybir.AluOpType.mult,
                    )
                return sel

            def store_rowmajor(w, colT_s, dram_dst, row_off, dt_out):
                rows = min(WIN, BLOCK - w * WIN)
                for h in range((rows + P - 1) // P):
                    rh = min(P, rows - h * P)
                    rm = ps_rm.tile([P, C_OUT], f32r, space="PSUM", tag="rm")
                    nc.tensor.transpose(
                        out=rm[:],
                        in_=colT_s[:, h * P:(h + 1) * P],
                        identity=w_s[0:C_OUT, 192:256],
                    )
                    rm_s = fpool.tile([P, C_OUT], dt_out, tag="rm_s")
                    nc.scalar.activation(out=rm_s[:], in_=rm[:].bitcast(f32), func=COPY)
                    r0 = w * WIN + h * P - row_off
                    nc.sync.dma_start(out=dram_dst[r0:r0 + rh, :], in_=rm_s[:rh, :])

            # ---------------- layer 1 ----------------
            import contextlib
            _es = contextlib.ExitStack()
            _rctx = contextlib.ExitStack()
            ra = _rctx.enter_context(nc.gpsimd.register("ra"))
            rb = _rctx.enter_context(nc.gpsimd.register("rb"))

            chunk_of = []
            acc_w = 0
            for k, cw in enumerate(CHUNK_WINS):
                chunk_of += [k] * cw

            gat1 = _es.enter_context(tc.tile_pool(name="gat1", bufs=GAT_BUFS))
            for w in range(NW):
                t_lo, t_hi = per_core0[w]["t1"]
                t_w = t_lo + t_hi
                msg = gat1.tile([P, t1max, C_IN], f32r, tag="msg1")
                if w < GAT_BUFS:
                    nc.vector.memset(msg[:].rearrange("p c e -> p (c e)").bitcast(f32), 0.0)
                ib = int(TB1[w]) * 8
                nc.gpsimd.reg_load(ra, cnt_s[0:1, 4 * w:4 * w + 1])
                nc.gpsimd.reg_load(rb, cnt_s[0:1, 4 * w + 1:4 * w + 2])
                if t_lo and not ABL_NO_GATHER:
                    nc.gpsimd.dma_gather(
                        out_ap=msg[:, 0:t_lo, :], in_ap=x_ext[:],
                        idxs_ap=i1_s[:, ib:ib + t_lo * 8],
                        num_idxs=t_lo * P, num_idxs_reg=ra, elem_size=C_IN,
                        single_packet=False, queue_num=(w % 2) * 2,
                    )
                if t_hi and not ABL_NO_GATHER:
                    nc.gpsimd.dma_gather(
                        out_ap=msg[:, t_lo:t_w, :], in_ap=x_ext[SPLIT:, :],
                        idxs_ap=i1_s[:, ib + t_lo * 8:ib + t_w * 8],
                        num_idxs=t_hi * P, num_idxs_reg=rb, elem_size=C_IN,
                        single_packet=False, queue_num=(w % 2) * 2 + 1,
                    )
                agg = ps_agg.tile([P, WIN], f32, space="PSUM", tag="agg")
                for t in range(1 if ABL_LITE_SCATTER else t_w):
                    col = int(TB1[w]) + t
                    sel = build_sel(col, c_dl1, c_nm1, c_ndl1, c_mnm1,
                                    on_act=(ACT_EVERY > 0 and col % ACT_EVERY == 0))
                    nc.tensor.matmul(
                        out=agg[:, :], lhsT=msg[:, t, :], rhs=sel[:],
                        start=(t == 0),
                        stop=(t == (0 if ABL_LITE_SCATTER else t_w - 1)),
                    )

                agg_s = wpool.tile([P, WIN], f32r, tag="agg_s")
                nc.scalar.activation(out=agg_s[:], in_=agg[:], func=COPY)
                z = ps_z.tile([P, WIN], f32, space="PSUM", tag="z")
                nc.tensor.matmul(out=z[:], lhsT=w_s[:, 0:C_HID], rhs=agg_s[:],
                                 start=True, stop=True)
                h1_s = wpool.tile([P, WIN], f32r, tag="h1")
                nc.scalar.activation(out=h1_s[:], in_=z[:], func=RELU,
                                     bias=meta_s[:, c_b1:c_b1 + 1])
                pt = ps_pt.tile([C_OUT, WIN], f32, space="PSUM", tag="pt")
                nc.tensor.matmul(out=pt[:], lhsT=w_s[:, 128:128 + C_OUT],
                                 rhs=h1_s[:], start=True, stop=True)
                pt_s = fpool.tile([C_OUT, WIN], f32r, tag="pt_s")
                nc.scalar.activation(out=pt_s[:], in_=pt[:], func=COPY)
                k = chunk_of[w]
                row_off = sum(CHUNK_ROWS[:k])
                store_rowmajor(w, pt_s, p_mine[k], row_off, f32r)

                # fire the allgather chunk as soon as its last window stored
                if (w == NW - 1 or chunk_of[w + 1] != k) and not ABL_NO_COLL:
                    pb = CHUNK_BASES[k] // 2
                    pr = CHUNK_ROWS[k] * CORES // 2
                    nc.gpsimd.collective_compute(
                        "AllGather", mybir.AluOpType.bypass,
                        replica_groups=[list(range(CORES))],
                        ins=[p_mine[k][:]], outs=[p_full[pb:pb + pr, :]],
                    )

            # ---------------- layer 2 ----------------
            _es.close()
            _es2 = contextlib.ExitStack()
            gat2 = _es2.enter_context(tc.tile_pool(name="gat2", bufs=GAT_BUFS))
            for w in range(NW):
                t_e, t_o = per_core0[w]["t2"]
                t_w = t_e + t_o
                msg = gat2.tile([P, t2max, 2 * C_OUT], f32r, tag="msg2")
                if w < GAT_BUFS:
                    nc.vector.memset(msg[:].rearrange("p c e -> p (c e)").bitcast(f32), 0.0)
                ib = int(TB2[w]) * 8
                nc.gpsimd.reg_load(ra, cnt_s[0:1, 4 * w + 2:4 * w + 3])
                nc.gpsimd.reg_load(rb, cnt_s[0:1, 4 * w + 3:4 * w + 4])
                if t_e and not ABL_NO_GATHER:
                    nc.gpsimd.dma_gather(
                        out_ap=msg[:, 0:t_e, :], in_ap=p_full[:],
                        idxs_ap=i2_s[:, ib:ib + t_e * 8],
                        num_idxs=t_e * P, num_idxs_reg=ra, elem_size=2 * C_OUT,
                        single_packet=False, queue_num=(w % 2) * 2,
                    )
                if t_o and not ABL_NO_GATHER:
                    nc.gpsimd.dma_gather(
                        out_ap=msg[:, t_e:t_w, :], in_ap=p_full[:],
                        idxs_ap=i2_s[:, ib + t_e * 8:ib + t_w * 8],
                        num_idxs=t_o * P, num_idxs_reg=rb, elem_size=2 * C_OUT,
                        single_packet=False, queue_num=(w % 2) * 2 + 1,
                    )
                agg = ps_agg.tile([P, WIN], f32, space="PSUM", tag="agg")
                for t in range(1 if ABL_LITE_SCATTER else t_w):
                    col = int(TB2[w]) + t
                    half = 0 if t < t_e else C_OUT
                    sel = build_sel(col, c_dl2, c_nm2, c_ndl2, c_mnm2,
                                    on_act=(ACT_EVERY > 0 and col % ACT_EVERY == 0))
                    nc.tensor.matmul(
                        out=agg[:C_OUT, :], lhsT=msg[:, t, half:half + C_OUT],
                        rhs=sel[:],
                        start=(t == 0),
                        stop=(t == (0 if ABL_LITE_SCATTER else t_w - 1)),
                    )

                o_s = fpool.tile([C_OUT, WIN], f32r, tag="o_s")
                nc.scalar.activation(out=o_s[:], in_=agg[:C_OUT, :], func=IDENT,
                                     bias=meta_s[0:C_OUT, c_b2:c_b2 + 1])
                store_rowmajor(w, o_s, out_ext, 0, f32)
            _es2.close()

    _rctx.close()
    nc.compile()
    layout = (c_dl1, c_nm1, c_ndl1, c_mnm1, c_dl2, c_nm2, c_ndl2, c_mnm2,
              c_iota, c_b1, c_b2, meta_cols, TB1, TB2)
    _BUILD_CACHE[sig] = (nc, layout)
    return nc, layout


def _uniformize(per_core):
    """Pad every core/window to the max tile counts so one program serves all
    cores (SPMD).  Returns the padded per-core data plus the shared structure."""
    uni = []
    for w in range(NW):
        t_lo = max(pc[w]["t1"][0] for pc in per_core)
        t_hi = max(pc[w]["t1"][1] for pc in per_core)
        t_e = max(pc[w]["t2"][0] for pc in per_core)
        t_o = max(pc[w]["t2"][1] for pc in per_core)
        uni.append({"t1": (t_lo, t_hi), "t2": (t_e, t_o)})

    padded = []
    for pc in per_core:
        wins = []
        for w in range(NW):
            src_w = pc[w]
            out_w = {"cnt1": src_w["cnt1"], "cnt2": src_w["cnt2"],
                     "t1": uni[w]["t1"], "t2": uni[w]["t2"]}
            for (layer, tkey) in (("1", "t1"), ("2", "t2")):
                ta_s, tb_s = src_w[tkey]
                ta_u, tb_u = uni[w][tkey]
                idx = np.full((ta_u + tb_u) * P, -1, np.int16)
                dl = np.full((ta_u + tb_u) * P, -1.0, np.float32)
                nm = np.zeros((ta_u + tb_u) * P, np.float32)
                s_idx, s_dl, s_nm = src_w["idx" + layer], src_w["dl" + layer], src_w["nm" + layer]
                idx[:ta_s * P] = s_idx[:ta_s * P]
                dl[:ta_s * P] = s_dl[:ta_s * P]
                nm[:ta_s * P] = s_nm[:ta_s * P]
                idx[ta_u * P:ta_u * P + tb_s * P] = s_idx[ta_s * P:]
                dl[ta_u * P:ta_u * P + tb_s * P] = s_dl[ta_s * P:]
                nm[ta_u * P:ta_u * P + tb_s * P] = s_nm[ta_s * P:]
                out_w["idx" + layer] = idx
                out_w["dl" + layer] = dl
                out_w["nm" + layer] = nm
            wins.append(out_w)
        padded.append(wins)
    return uni, padded


def _make_inputs(x, W1, b1, W2, b2, uni, padded, layout):
    (c_dl1, c_nm1, c_ndl1, c_mnm1, c_dl2, c_nm2, c_ndl2, c_mnm2,
     c_iota, c_b1, c_b2, meta_cols, TB1, TB2) = layout
    tot1, tot2 = int(TB1[-1]), int(TB2[-1])

    wts = np.zeros((P, 256), np.float32)
    wts[:, 0:128] = W1
    wts[:128, 128:192] = W2
    wts[0:64, 192:256] = np.eye(64, dtype=np.float32)

    def lanes(flat):  # [T*P] -> [P, T] (tile t, lane p) at column t
        return flat.reshape(-1, P).T

    in_maps = []
    for pc in padded:
        idx1 = np.concatenate([pc[w]["idx1"] for w in range(NW)])
        idx2 = np.concatenate([pc[w]["idx2"] for w in range(NW)])
        dl1 = np.concatenate([pc[w]["dl1"] for w in range(NW)])
        nm1 = np.concatenate([pc[w]["nm1"] for w in range(NW)])
        dl2 = np.concatenate([pc[w]["dl2"] for w in range(NW)])
        nm2 = np.concatenate([pc[w]["nm2"] for w in range(NW)])

        meta = np.zeros((P, meta_cols), np.float32)
        meta[:, c_dl1:c_dl1 + tot1] = lanes(dl1)
        meta[:, c_nm1:c_nm1 + tot1] = lanes(nm1)
        meta[:, c_ndl1:c_ndl1 + tot1] = -lanes(dl1)
        meta[:, c_mnm1:c_mnm1 + tot1] = -lanes(nm1)
        meta[:, c_dl2:c_dl2 + tot2] = lanes(dl2)
        meta[:, c_nm2:c_nm2 + tot2] = lanes(nm2)
        meta[:, c_ndl2:c_ndl2 + tot2] = -lanes(dl2)
        meta[:, c_mnm2:c_mnm2 + tot2] = -lanes(nm2)
        meta[:, c_iota:c_iota + WIN] = np.arange(WIN, dtype=np.float32)[None, :]
        meta[:, c_b1] = b1
        meta[:C_OUT, c_b2] = b2

        cnt = np.zeros((NW, 4), np.int32)
        for w in range(NW):
            cnt[w, 0], cnt[w, 1] = pc[w]["cnt1"]
            cnt[w, 2], cnt[w, 3] = pc[w]["cnt2"]

        in_maps.append({
            "x": np.ascontiguousarray(x, dtype=np.float32),
            "idx1": _wrap16(idx1),
            "idx2": _wrap16(idx2),
            "meta": meta,
            "wts": wts,
            "cnt": cnt.reshape(1, -1),
        })
    return in_maps


def kernel(x, edge_index, W1, b1, W2, b2):
    x = np.asarray(x, dtype=np.float32)
    W1 = np.asarray(W1, dtype=np.float32)
    b1 = np.asarray(b1, dtype=np.float32)
    W2 = np.asarray(W2, dtype=np.float32)
    b2 = np.asarray(b2, dtype=np.float32)

    sig, per_core = _preprocess(np.asarray(edge_index))
    uni, padded = _uniformize(per_core)
    usig = tuple((u["t1"], u["t2"]) for u in uni)
    nc, layout = _build(usig, uni)
    in_maps = _make_inputs(x, W1, b1, W2, b2, uni, padded, layout)
    res = run_bass_kernel_spmd(nc, in_maps, list(range(CORES)))
    out = np.concatenate([res.results[c]["out"] for c in range(CORES)], axis=0)
    return out.astype(np.float32)


# revision 26
# speedup vs baseline: 1.1111x; 1.0059x over previous
"""2-layer GCN (PyG GCNConv semantics) on 8 Trainium2 NeuronCores.

Strategy (dst-sharded message passing):
  - Nodes are split into 8 contiguous blocks of 6250 rows; core c owns output
    rows [6250c, 6250(c+1)).  Edges (plus one self-loop per node) are
    partitioned by destination core, then by 256-node destination windows,
    then packed into 128-edge tiles with EXACT per-window tile counts
    (uniformized across cores so one SPMD program serves all 8).
  - Layer 1: per window two dma_gather calls (lo: src < 32768 on the base x
    table, hi: offset view; int16 index reach), 512B descriptors.  Window
    pairs alternate SWDGE queue pairs {0,1}/{2,3} so all four Q7 queue-pair
    workers generate descriptors concurrently (measured 512B floor ~3.3
    ns/row on 4 queues vs ~8.2 on one; 256B descriptors are slower, ~4.1).
    Tile padding is trailing -1 indices with num_idxs_reg = the real count.
    NOTE: never rely on the Q7-side trailing-(-1) trim with a larger count
    and never put -1 mid-stream -- both break the DMA completion semaphore
    protocol and hang the device (mid-stream padding must use idx=0).
  - Scatter: per edge tile a norm-scaled one-hot dst matrix
    (sel = (iota == dst_local) * norm; padded lanes have dst_local=-1) feeds
    aggT[feat, dst] += msg^T @ sel on the PE in float32r.  sel tiles are
    built on the DVE (tensor_scalar, 2x_2p), except every ACT_EVERY-th tile
    which is built on the Activation engine in two ops
    (u = Abs(iota - dloc); sel = Relu(-nrm*u + nrm)) to relieve the DVE --
    the busiest non-gather engine -- and its SBUF-port contention with the
    Q7 descriptor writers.
  - Window flush: h1T = relu(W1^T @ aggT + b1); pT = W2^T @ h1T; PE-transpose
    to row-major and store p rows (f32r) to DRAM; one AllGather of p
    (12.8 MB, ~107 us serial exposure; chunking it into the layer-1 tail
    measured slower -- the collective blocks the Pool queue mid-layer).
  - Layer 2 gathers PAIRS of 256-byte p rows as single 512-byte descriptors
    (p_full viewed as [25000, 128] f32): pairing sidesteps the slower 256B
    descriptor rate and the int16 lo/hi split.  Edge tiles are segregated by
    source-row parity; the scatter matmul reads the matching 64-column half
    of the gathered pair.

Host-side work is index preprocessing only (degrees/norms from edge_index,
sorting, packing); all FLOPs on the gathered/aggregated features run on
device.  Measured: 2.06 ms (previous baseline) -> 1.75 ms.
"""
import os
import sys

import numpy as np

for _p in ("/opt/trn_rl_repo", "/root/.axon_site/_ro/trn_rl_repo"):
    if os.path.isdir(_p) and _p not in sys.path:
        sys.path.insert(0, _p)

import concourse.bacc as bacc
import concourse.tile as tile
from concourse import mybir
from concourse.bass_utils import run_bass_kernel_spmd

P = 128
N_NODES = 50000
C_IN = 128
C_HID = 128
C_OUT = 64
CORES = 8
BLOCK = N_NODES // CORES          # 6250
WIN = 256                         # dst nodes per PSUM window
NW = -(-BLOCK // WIN)             # 25 windows per core (last has 106 rows)
SPLIT = 32768                     # int16-addressable base-table rows
GAT_BUFS = 6                      # msg tile buffering depth
ACT_EVERY = 3                     # every Nth sel tile on Act (0 = off)
ABL_LITE_SCATTER = bool(int(os.environ.get("ABL_LITE_SCATTER", "0")))
ABL_NO_GATHER = bool(int(os.environ.get("ABL_NO_GATHER", "0")))
ABL_NO_COLL = bool(int(os.environ.get("ABL_NO_COLL", "0")))
CHUNK_WINS = (NW,)               # allgather chunks


def _chunk_geometry():
    rows = []
    r0 = 0
    bases = []
    base = 0
    for cw in CHUNK_WINS:
        rk = min(BLOCK - r0, cw * WIN)
        rows.append(rk)
        bases.append(base)
        base += rk * CORES
        r0 += rk
    return rows, bases


CHUNK_ROWS, CHUNK_BASES = _chunk_geometry()


def _tab_map():
    """node id -> chunk-major row in p_full ([chunk][core][local row])."""
    n = np.arange(N_NODES, dtype=np.int64)
    q = n // BLOCK
    r = n % BLOCK
    tab = np.empty(N_NODES, dtype=np.int64)
    r0 = 0
    for rk, base in zip(CHUNK_ROWS, CHUNK_BASES):
        m = (r >= r0) & (r < r0 + rk)
        tab[m] = base + q[m] * rk + (r[m] - r0)
        r0 += rk
    return tab


def _pack_tiles(vals_idx, vals_dloc, vals_nrm, n_tiles):
    """Pad a call's edge list to n_tiles*128 lanes.  Padding gathers row 0
    (idx=0, a real address -- keeps every msg lane finite without memsets)
    and is killed in the selection matrix via dloc=-1 / nrm=0."""
    n = len(vals_idx)
    idx = np.full(n_tiles * P, -1, np.int16)
    dl = np.full(n_tiles * P, -1.0, np.float32)
    nm = np.zeros(n_tiles * P, np.float32)
    idx[:n] = vals_idx
    dl[:n] = vals_dloc
    nm[:n] = vals_nrm
    return idx, dl, nm


def _wrap16(a):
    """[L] int16 idx array -> [128, L/16] wrapped+replicated layout."""
    w16 = a.reshape(-1, 16).T.reshape(16, -1)
    return np.tile(w16, (8, 1))


def _preprocess(edge_index):
    src = np.asarray(edge_index[0], dtype=np.int64)
    dst = np.asarray(edge_index[1], dtype=np.int64)

    deg = np.bincount(dst, minlength=N_NODES).astype(np.float64) + 1.0
    dinv = 1.0 / np.sqrt(deg)

    a_src = np.concatenate([src, np.arange(N_NODES, dtype=np.int64)])
    a_dst = np.concatenate([dst, np.arange(N_NODES, dtype=np.int64)])
    a_nrm = np.concatenate([dinv[src] * dinv[dst], dinv * dinv]).astype(np.float32)

    tab = _tab_map()
    core = a_dst // BLOCK
    win = (a_dst % BLOCK) // WIN
    dloc = ((a_dst % BLOCK) % WIN).astype(np.float32)

    per_core = []
    for c in range(CORES):
        wins = []
        for w in range(NW):
            m = (core == c) & (win == w)
            s = a_src[m]
            d = dloc[m]
            nm = a_nrm[m]

            # layer 1: split by int16 addressability
            lo = s < SPLIT
            n_lo, n_hi = int(lo.sum()), int((~lo).sum())
            t_lo, t_hi = -(-n_lo // P), -(-n_hi // P)
            i_lo, dl_lo, nm_lo = _pack_tiles(s[lo].astype(np.int16), d[lo], nm[lo], t_lo)
            i_hi, dl_hi, nm_hi = _pack_tiles((s[~lo] - SPLIT).astype(np.int16),
                                             d[~lo], nm[~lo], t_hi)

            # layer 2: pair-gather from chunk-major table; split by parity
            t2 = tab[s]
            ev = (t2 & 1) == 0
            n_e, n_o = int(ev.sum()), int((~ev).sum())
            t_e, t_o = -(-n_e // P), -(-n_o // P)
            i_e, dl_e, nm_e = _pack_tiles((t2[ev] >> 1).astype(np.int16), d[ev], nm[ev], t_e)
            i_o, dl_o, nm_o = _pack_tiles((t2[~ev] >> 1).astype(np.int16), d[~ev], nm[~ev], t_o)

            wins.append({
                "cnt1": (n_lo, n_hi), "t1": (t_lo, t_hi),
                "idx1": np.concatenate([i_lo, i_hi]),
                "dl1": np.concatenate([dl_lo, dl_hi]),
                "nm1": np.concatenate([nm_lo, nm_hi]),
                "cnt2": (n_e, n_o), "t2": (t_e, t_o),
                "idx2": np.concatenate([i_e, i_o]),
                "dl2": np.concatenate([dl_e, dl_o]),
                "nm2": np.concatenate([nm_e, nm_o]),
            })
        per_core.append(wins)

    sig = tuple(
        (wins[w]["t1"], wins[w]["t2"]) for wins in per_core for w in range(NW)
    )
    return sig, per_core


_BUILD_CACHE = {}


def _build(sig, per_core0):
    """per_core0: any core's window list -- only tile-count structure is used,
    and it must be identical across cores, so callers pass the max-shaped
    structure via _uniformize."""
    if sig in _BUILD_CACHE:
        return _BUILD_CACHE[sig]

    # per-window tile counts (uniform across cores after _uniformize)
    T1 = [sum(w["t1"]) for w in per_core0]
    T2 = [sum(w["t2"]) for w in per_core0]
    TB1 = np.concatenate([[0], np.cumsum(T1)]).astype(int)   # tile col bases
    TB2 = np.concatenate([[0], np.cumsum(T2)]).astype(int)
    tot1, tot2 = int(TB1[-1]), int(TB2[-1])
    t1max, t2max = max(T1), max(T2)

    # meta f32 columns:
    # [dl1 | nm1 | ndl1 | mnm1 | dl2 | nm2 | ndl2 | mnm2 | iota(WIN) | b1 | b2]
    c_dl1, c_nm1, c_ndl1, c_mnm1 = 0, tot1, 2 * tot1, 3 * tot1
    base2 = 4 * tot1
    c_dl2, c_nm2 = base2, base2 + tot2
    c_ndl2, c_mnm2 = base2 + 2 * tot2, base2 + 3 * tot2
    c_iota = base2 + 4 * tot2
    c_b1, c_b2 = c_iota + WIN, c_iota + WIN + 1
    meta_cols = c_iota + WIN + 2

    f32, f32r, i16 = mybir.dt.float32, mybir.dt.float32r, mybir.dt.int16
    RELU = mybir.ActivationFunctionType.Relu
    COPY = mybir.ActivationFunctionType.Copy
    IDENT = mybir.ActivationFunctionType.Identity

    nc = bacc.Bacc("TRN2", num_devices=CORES, num_swdge_queues=4)
    x_ext = nc.dram_tensor("x", [N_NODES, C_IN], f32r, kind="ExternalInput")
    i1_ext = nc.dram_tensor("idx1", [P, tot1 * 8], i16, kind="ExternalInput")
    i2_ext = nc.dram_tensor("idx2", [P, tot2 * 8], i16, kind="ExternalInput")
    meta_ext = nc.dram_tensor("meta", [P, meta_cols], f32, kind="ExternalInput")
    cnt_ext = nc.dram_tensor("cnt", [1, NW * 4], mybir.dt.int32, kind="ExternalInput")
    w_ext = nc.dram_tensor("wts", [P, 256], f32r, kind="ExternalInput")
    out_ext = nc.dram_tensor("out", [BLOCK, C_OUT], f32, kind="ExternalOutput")

    with tile.TileContext(nc) as tc:
        with tc.tile_pool(name="const", bufs=1) as cpool, \
             tc.tile_pool(name="work", bufs=4) as wpool, \
             tc.tile_pool(name="flush", bufs=3) as fpool, \
             tc.tile_pool(name="dram", bufs=1, space="DRAM") as dpool, \
             tc.tile_pool(name="ps_agg", bufs=2, space="PSUM") as ps_agg, \
             tc.tile_pool(name="ps_z", bufs=2, space="PSUM") as ps_z, \
             tc.tile_pool(name="ps_pt", bufs=2, space="PSUM") as ps_pt, \
             tc.tile_pool(name="ps_rm", bufs=2, space="PSUM") as ps_rm:

            i1_s = cpool.tile([P, tot1 * 8], i16)
            i2_s = cpool.tile([P, tot2 * 8], i16)
            meta_s = cpool.tile([P, meta_cols], f32)
            w_s = cpool.tile([P, 256], f32r)
            cnt_s = cpool.tile([1, NW * 4], mybir.dt.int32)
            nc.sync.dma_start(out=cnt_s[:], in_=cnt_ext[:])
            nc.sync.dma_start(out=i1_s[:], in_=i1_ext[:])
            nc.sync.dma_start(out=i2_s[:], in_=i2_ext[:])
            nc.sync.dma_start(out=meta_s[:], in_=meta_ext[:])
            nc.sync.dma_start(out=w_s[:], in_=w_ext[:])

            p_mine = [dpool.tile([rk, C_OUT], f32r, name=f"p_mine{rk}")
                      for rk in CHUNK_ROWS]
            p_full = dpool.tile([N_NODES // 2, 2 * C_OUT], f32r)

            def build_sel(col, c_dl, c_nm, c_ndl, c_mnm, on_act):
                sel = wpool.tile([P, WIN], f32r, tag="sel")
                if on_act:
                    u = wpool.tile([P, WIN], f32, tag="selu")
                    nc.scalar.activation(
                        out=u[:], in_=meta_s[:, c_iota:c_iota + WIN],
                        func=mybir.ActivationFunctionType.Abs,
                        bias=meta_s[:, c_ndl + col:c_ndl + col + 1])
                    nc.scalar.activation(
                        out=sel[:], in_=u[:], func=RELU,
                        scale=meta_s[:, c_mnm + col:c_mnm + col + 1],
                        bias=meta_s[:, c_nm + col:c_nm + col + 1])
                else:
                    nc.vector.tensor_scalar(
                        out=sel[:],
                        in0=meta_s[:, c_iota:c_iota + WIN],
                        scalar1=meta_s[:, c_dl + col:c_dl + col + 1],
                        scalar2=meta_s[:, c_nm + col:c_nm + col + 1],
                        op0=mybir.AluOpType.is_equal,
                        op1=mybir.AluOpType.mult,
                    )
                return sel

            def store_rowmajor(w, colT_s, dram_dst, row_off, dt_out):
                rows = min(WIN, BLOCK - w * WIN)
                for h in range((rows + P - 1) // P):
                    rh = min(P, rows - h * P)
                    rm = ps_rm.tile([P, C_OUT], f32r, space="PSUM", tag="rm")
                    nc.tensor.transpose(
                        out=rm[:],
                        in_=colT_s[:, h * P:(h + 1) * P],
                        identity=w_s[0:C_OUT, 192:256],
                    )
                    rm_s = fpool.tile([P, C_OUT], dt_out, tag="rm_s")
                    nc.scalar.activation(out=rm_s[:], in_=rm[:].bitcast(f32), func=COPY)
                    r0 = w * WIN + h * P - row_off
                    nc.sync.dma_start(out=dram_dst[r0:r0 + rh, :], in_=rm_s[:rh, :])

            # ---------------- layer 1 ----------------
            import contextlib
            _es = contextlib.ExitStack()
            _rctx = contextlib.ExitStack()
            ra = _rctx.enter_context(nc.gpsimd.register("ra"))
            rb = _rctx.enter_context(nc.gpsimd.register("rb"))

            chunk_of = []
            acc_w = 0
            for k, cw in enumerate(CHUNK_WINS):
                chunk_of += [k] * cw

            gat1 = _es.enter_context(tc.tile_pool(name="gat1", bufs=GAT_BUFS))
            for w in range(NW):
                t_lo, t_hi = per_core0[w]["t1"]
                t_w = t_lo + t_hi
                msg = gat1.tile([P, t1max, C_IN], f32r, tag="msg1")
                if w < GAT_BUFS:
                    nc.vector.memset(msg[:].rearrange("p c e -> p (c e)").bitcast(f32), 0.0)
                ib = int(TB1[w]) * 8
                nc.gpsimd.reg_load(ra, cnt_s[0:1, 4 * w:4 * w + 1])
                nc.gpsimd.reg_load(rb, cnt_s[0:1, 4 * w + 1:4 * w + 2])
                if t_lo and not ABL_NO_GATHER:
                    nc.gpsimd.dma_gather(
                        out_ap=msg[:, 0:t_lo, :], in_ap=x_ext[:],
                        idxs_ap=i1_s[:, ib:ib + t_lo * 8],
                        num_idxs=t_lo * P, num_idxs_reg=ra, elem_size=C_IN,
                        single_packet=False, queue_num=(w % 2) * 2,
                    )
                if t_hi and not ABL_NO_GATHER:
                    nc.gpsimd.dma_gather(
                        out_ap=msg[:, t_lo:t_w, :], in_ap=x_ext[SPLIT:, :],
                        idxs_ap=i1_s[:, ib + t_lo * 8:ib + t_w * 8],
                        num_idxs=t_hi * P, num_idxs_reg=rb, elem_size=C_IN,
                        single_packet=False, queue_num=(w % 2) * 2 + 1,
                    )
                agg = ps_agg.tile([P, WIN], f32, space="PSUM", tag="agg")
                for t in range(1 if ABL_LITE_SCATTER else t_w):
                    col = int(TB1[w]) + t
                    sel = build_sel(col, c_dl1, c_nm1, c_ndl1, c_mnm1,
                                    on_act=(ACT_EVERY > 0 and col % ACT_EVERY == 0))
                    nc.tensor.matmul(
                        out=agg[:, :], lhsT=msg[:, t, :], rhs=sel[:],
                        start=(t == 0),
                        stop=(t == (0 if ABL_LITE_SCATTER else t_w - 1)),
                    )

                agg_s = wpool.tile([P, WIN], f32r, tag="agg_s")
                nc.scalar.activation(out=agg_s[:], in_=agg[:], func=COPY)
                z = ps_z.tile([P, WIN], f32, space="PSUM", tag="z")
                nc.tensor.matmul(out=z[:], lhsT=w_s[:, 0:C_HID], rhs=agg_s[:],
                                 start=True, stop=True)
                h1_s = wpool.tile([P, WIN], f32r, tag="h1")
                nc.scalar.activation(out=h1_s[:], in_=z[:], func=RELU,
                                     bias=meta_s[:, c_b1:c_b1 + 1])
                pt = ps_pt.tile([C_OUT, WIN], f32, space="PSUM", tag="pt")
                nc.tensor.matmul(out=pt[:], lhsT=w_s[:, 128:128 + C_OUT],
                                 rhs=h1_s[:], start=True, stop=True)
                pt_s = fpool.tile([C_OUT, WIN], f32r, tag="pt_s")
                nc.scalar.activation(out=pt_s[:], in_=pt[:], func=COPY)
                k = chunk_of[w]
                row_off = sum(CHUNK_ROWS[:k])
                store_rowmajor(w, pt_s, p_mine[k], row_off, f32r)

                # fire the allgather chunk as soon as its last window stored
                if (w == NW - 1 or chunk_of[w + 1] != k) and not ABL_NO_COLL:
                    pb = CHUNK_BASES[k] // 2
                    pr = CHUNK_ROWS[k] * CORES // 2
                    nc.gpsimd.collective_compute(
                        "AllGather", mybir.AluOpType.bypass,
                        replica_groups=[list(range(CORES))],
                        ins=[p_mine[k][:]], outs=[p_full[pb:pb + pr, :]],
                    )

            # ---------------- layer 2 ----------------
            _es.close()
            _es2 = contextlib.ExitStack()
            gat2 = _es2.enter_context(tc.tile_pool(name="gat2", bufs=GAT_BUFS))
            for w in range(NW):
                t_e, t_o = per_core0[w]["t2"]
                t_w = t_e + t_o
                msg = gat2.tile([P, t2max, 2 * C_OUT], f32r, tag="msg2")
                if w < GAT_BUFS:
                    nc.vector.memset(msg[:].rearrange("p c e -> p (c e)").bitcast(f32), 0.0)
                ib = int(TB2[w]) * 8
                nc.gpsimd.reg_load(ra, cnt_s[0:1, 4 * w + 2:4 * w + 3])
                nc.gpsimd.reg_load(rb, cnt_s[0:1, 4 * w + 3:4 * w + 4])
                if t_e and not ABL_NO_GATHER:
                    nc.gpsimd.dma_gather(
                        out_ap=msg[:, 0:t_e, :], in_ap=p_full[:],
                        idxs_ap=i2_s[:, ib:ib + t_e * 8],
                        num_idxs=t_e * P, num_idxs_reg=ra, elem_size=2 * C_OUT,
                        single_packet=False, queue_num=(w % 2) * 2,
                    )
                if t_o and not ABL_NO_GATHER:
                    nc.gpsimd.dma_gather(
                        out_ap=msg[:, t_e:t_w, :], in_ap=p_full[:],
                        idxs_ap=i2_s[:, ib + t_e * 8:ib + t_w * 8],
                        num_idxs=t_o * P, num_idxs_reg=rb, elem_size=2 * C_OUT,
                        single_packet=False, queue_num=(w % 2) * 2 + 1,
                    )
                agg = ps_agg.tile([P, WIN], f32, space="PSUM", tag="agg")
                for t in range(1 if ABL_LITE_SCATTER else t_w):
                    col = int(TB2[w]) + t
                    half = 0 if t < t_e else C_OUT
                    sel = build_sel(col, c_dl2, c_nm2, c_ndl2, c_mnm2,
                                    on_act=(ACT_EVERY > 0 and col % ACT_EVERY == 0))
                    nc.tensor.matmul(
                        out=agg[:C_OUT, :], lhsT=msg[:, t, half:half + C_OUT],
                        rhs=sel[:],
                        start=(t == 0),
                        stop=(t == (0 if ABL_LITE_SCATTER else t_w - 1)),
                    )

                o_s = fpool.tile([C_OUT, WIN], f32r, tag="o_s")
                nc.scalar.activation(out=o_s[:], in_=agg[:C_OUT, :], func=IDENT,
                                     bias=meta_s[0:C_OUT, c_b2:c_b2 + 1])
                store_rowmajor(w, o_s, out_ext, 0, f32)
            _es2.close()

    _rctx.close()
    nc.compile()
    layout = (c_dl1, c_nm1, c_ndl1, c_mnm1, c_dl2, c_nm2, c_ndl2, c_mnm2,
              c_iota, c_b1, c_b2, meta_cols, TB1, TB2)
    _BUILD_CACHE[sig] = (nc, layout)
    return nc, layout


def _uniformize(per_core):
    """Pad every core/window to the max tile counts so one program serves all
    cores (SPMD).  Returns the padded per-core data plus the shared structure."""
    uni = []
    for w in range(NW):
        t_lo = max(pc[w]["t1"][0] for pc in per_core)
        t_hi = max(pc[w]["t1"][1] for pc in per_core)
        t_e = max(pc[w]["t2"][0] for pc in per_core)
        t_o = max(pc[w]["t2"][1] for pc in per_core)
        uni.append({"t1": (t_lo, t_hi), "t2": (t_e, t_o)})

    padded = []
    for pc in per_core:
        wins = []
        for w in range(NW):
            src_w = pc[w]
            out_w = {"cnt1": src_w["cnt1"], "cnt2": src_w["cnt2"],
                     "t1": uni[w]["t1"], "t2": uni[w]["t2"]}
            for (layer, tkey) in (("1", "t1"), ("2", "t2")):
                ta_s, tb_s = src_w[tkey]
                ta_u, tb_u = uni[w][tkey]
                idx = np.full((ta_u + tb_u) * P, -1, np.int16)
                dl = np.full((ta_u + tb_u) * P, -1.0, np.float32)
                nm = np.zeros((ta_u + tb_u) * P, np.float32)
                s_idx, s_dl, s_nm = src_w["idx" + layer], src_w["dl" + layer], src_w["nm" + layer]
                idx[:ta_s * P] = s_idx[:ta_s * P]
                dl[:ta_s * P] = s_dl[:ta_s * P]
                nm[:ta_s * P] = s_nm[:ta_s * P]
                idx[ta_u * P:ta_u * P + tb_s * P] = s_idx[ta_s * P:]
                dl[ta_u * P:ta_u * P + tb_s * P] = s_dl[ta_s * P:]
                nm[ta_u * P:ta_u * P + tb_s * P] = s_nm[ta_s * P:]
                out_w["idx" + layer] = idx
                out_w["dl" + layer] = dl
                out_w["nm" + layer] = nm
            wins.append(out_w)
        padded.append(wins)
    return uni, padded


def _make_inputs(x, W1, b1, W2, b2, uni, padded, layout):
    (c_dl1, c_nm1, c_ndl1, c_mnm1, c_dl2, c_nm2, c_ndl2, c_mnm2,
     c_iota, c_b1, c_b2, meta_cols, TB1, TB2) = layout
    tot1, tot2 = int(TB1[-1]), int(TB2[-1])

    wts = np.zeros((P, 256), np.float32)
    wts[:, 0:128] = W1
    wts[:128, 128:192] = W2
    wts[0:64, 192:256] = np.eye(64, dtype=np.float32)

    def lanes(flat):  # [T*P] -> [P, T] (tile t, lane p) at column t
        return flat.reshape(-1, P).T

    in_maps = []
    for pc in padded:
        idx1 = np.concatenate([pc[w]["idx1"] for w in range(NW)])
        idx2 = np.concatenate([pc[w]["idx2"] for w in range(NW)])
        dl1 = np.concatenate([pc[w]["dl1"] for w in range(NW)])
        nm1 = np.concatenate([pc[w]["nm1"] for w in range(NW)])
        dl2 = np.concatenate([pc[w]["dl2"] for w in range(NW)])
        nm2 = np.concatenate([pc[w]["nm2"] for w in range(NW)])

        meta = np.zeros((P, meta_cols), np.float32)
        meta[:, c_dl1:c_dl1 + tot1] = lanes(dl1)
        meta[:, c_nm1:c_nm1 + tot1] = lanes(nm1)
        meta[:, c_ndl1:c_ndl1 + tot1] = -lanes(dl1)
        meta[:, c_mnm1:c_mnm1 + tot1] = -lanes(nm1)
        meta[:, c_dl2:c_dl2 + tot2] = lanes(dl2)
        meta[:, c_nm2:c_nm2 + tot2] = lanes(nm2)
        meta[:, c_ndl2:c_ndl2 + tot2] = -lanes(dl2)
        meta[:, c_mnm2:c_mnm2 + tot2] = -lanes(nm2)
        meta[:, c_iota:c_iota + WIN] = np.arange(WIN, dtype=np.float32)[None, :]
        meta[:, c_b1] = b1
        meta[:C_OUT, c_b2] = b2

        cnt = np.zeros((NW, 4), np.int32)
        for w in range(NW):
            cnt[w, 0], cnt[w, 1] = pc[w]["cnt1"]
            cnt[w, 2], cnt[w, 3] = pc[w]["cnt2"]

        in_maps.append({
            "x": np.ascontiguousarray(x, dtype=np.float32),
            "idx1": _wrap16(idx1),
            "idx2": _wrap16(idx2),
            "meta": meta,
            "wts": wts,
            "cnt": cnt.reshape(1, -1),
        })
    return in_maps


def kernel(x, edge_index, W1, b1, W2, b2):
    x = np.asarray(x, dtype=np.float32)
    W1 = np.asarray(W1, dtype=np.float32)
    b1 = np.asarray(b1, dtype=np.float32)
    W2 = np.asarray(W2, dtype=np.float32)
    b2 = np.asarray(b2, dtype=np.float32)

    sig, per_core = _preprocess(np.asarray(edge_index))
    uni, padded = _uniformize(per_core)
    usig = tuple((u["t1"], u["t2"]) for u in uni)
    nc, layout = _build(usig, uni)
    in_maps = _make_inputs(x, W1, b1, W2, b2, uni, padded, layout)
    res = run_bass_kernel_spmd(nc, in_maps, list(range(CORES)))
    out = np.concatenate([res.results[c]["out"] for c in range(CORES)], axis=0)
    return out.astype(np.float32)
